# revision 13
# baseline (speedup 1.0000x reference)
"""Trainium2 Bass kernel for the YAT MixerBlock (nn_MixerBlock_12524124635797).

Strategy: pure data-parallel over batch (64 -> 8 per core). Each core runs
the full mixer block for its 8 batch elements.

Per-core dataflow (all GEMMs fp16 inputs, fp32 PSUM accumulation):
  Token stage (per batch b, x_b is (196p, 768c)):
    dot1 (384t-part, 768c-free) = twT.T @ x_b            [PE]
    den  = wn_t[t] + xn[c] - 2*dot1 + eps                [DVE affine_then_add]
    rec  = 1/den                                         [DVE reciprocal_approx_fast]
    sq   = (dot1 + tb[t])^2                              [ACT Square, bias slot]
    h1   = sq * rec  (fp16)                              [GPSIMD mult; scale_t folded into w2]
    x2T (768c-part, 196p-free) = h1.T@w2sT + x_b.T@I196 + ones.T@b2row   [PE, shortcut+bias
                                                          folded in as extra K rows]
  Channel stage (rows = (b,p) flattened, 1568 per core):
    xn2b (128, rows) = ones.T @ (x2T*x2T)                [PE broadcast of row norms]
    for row-block rb, for m-chunk mc (24 chunks of 3072):
      dot2 (128m-part, rows-free) = cwT.T @ x2T          [PE]
      den2/rec2/sq2/h2 as above (wn_c, cb per-partition) [DVE/ACT/GPSIMD]
      out_psum(rows-part, 768c) += h2.T @ w4sT[mc]       [PE]
    out_psum += x2T.T @ I768 + ones.T @ b4row            [PE, shortcut+bias]
    out (rows, 768) fp32 -> DRAM                         [ACT copy + DMA]
"""

import numpy as np

import concourse.bass as bass
import concourse.bacc as bacc
import concourse.mybir as mybir
from concourse import bass_utils
from concourse import tile

F16 = mybir.dt.float16
F32 = mybir.dt.float32
AF = mybir.ActivationFunctionType

EPS = 0.1
B, P, C, T, M3 = 64, 196, 768, 384, 3072
NCORES = 8
BL = B // NCORES          # 8 batches per core
ROWS = BL * P             # 1568 rows per core
RB = 256                  # row-block size for the channel stage (2 psum chunks)


def _ceil_div(a, b):
    return (a + b - 1) // b


def _n_slices(n, step=512):
    """Split [0, n) into matmul-legal free-dim slices (<=512, bank-aligned)."""
    out = []
    o = 0
    while o < n:
        out.append((o, min(step, n - o)))
        o += step
    return out


def build_program():
    nc = bacc.Bacc(
        "TRN2",
        target_bir_lowering=False,
        debug=False,
        enable_asserts=False,
        num_devices=NCORES,
    )

    # ---- DRAM I/O ----
    d = {}
    d["x16"] = nc.dram_tensor("x16", [ROWS, C], F16, kind="ExternalInput").ap()
    d["twT"] = nc.dram_tensor("twT", [128, 2, T], F16, kind="ExternalInput").ap()
    d["w2sT"] = nc.dram_tensor("w2sT", [128, 3, P], F16, kind="ExternalInput").ap()
    d["i196"] = nc.dram_tensor("i196", [128, 2, P], F16, kind="ExternalInput").ap()
    d["b2r"] = nc.dram_tensor("b2r", [1, P], F16, kind="ExternalInput").ap()
    d["cwT"] = nc.dram_tensor("cwT", [128, 6, M3], F16, kind="ExternalInput").ap()
    d["w4sT"] = nc.dram_tensor("w4sT", [128, 24, C], F16, kind="ExternalInput").ap()
    d["i768"] = nc.dram_tensor("i768", [128, 6, C], F16, kind="ExternalInput").ap()
    d["b4r"] = nc.dram_tensor("b4r", [1, C], F16, kind="ExternalInput").ap()
    d["wnt"] = nc.dram_tensor("wnt", [128, 3], F32, kind="ExternalInput").ap()
    d["tbc"] = nc.dram_tensor("tbc", [128, 3], F32, kind="ExternalInput").ap()
    d["wnc"] = nc.dram_tensor("wnc", [128, 24], F32, kind="ExternalInput").ap()
    d["cbc"] = nc.dram_tensor("cbc", [128, 24], F32, kind="ExternalInput").ap()
    out_dram = nc.dram_tensor("out", [ROWS, C], F32, kind="ExternalOutput").ap()

    with tile.TileContext(nc) as tc:
        with tc.tile_pool(name="consts", bufs=1) as cp:
            # Resident constants / persistent activations.
            twT = cp.tile([128, 2, T], F16)
            w2sT = cp.tile([128, 3, P], F16)
            i196 = cp.tile([128, 2, P], F16)
            b2r = cp.tile([128, P], F16)
            cwT = cp.tile([128, 6, M3], F16)
            w4sT = cp.tile([128, 24, C], F16)
            i768 = cp.tile([128, 6, C], F16)
            b4r = cp.tile([128, C], F16)
            wnt = cp.tile([128, 3], F32)
            tbc = cp.tile([128, 3], F32)
            wnc = cp.tile([128, 24], F32)
            cbc = cp.tile([128, 24], F32)
            ones = cp.tile([128, 128], F16)
            x2T = cp.tile([128, 6, ROWS], F16)
            xn2b = cp.tile([128, ROWS], F32)

            # Token-critical constants on the sync queue (small, arrive first);
            # the big channel weights go on the scalar-engine HWDGE queue so
            # they don't block the token stage's x DMAs.
            nc.sync.dma_start(twT[:], d["twT"])
            nc.sync.dma_start(w2sT[:], d["w2sT"])
            nc.sync.dma_start(i196[:], d["i196"])
            nc.sync.dma_start(b2r[0:1, :], d["b2r"])
            nc.sync.dma_start(wnt[:], d["wnt"])
            nc.sync.dma_start(tbc[:], d["tbc"])
            nc.sync.dma_start(wnc[:], d["wnc"])
            nc.sync.dma_start(cbc[:], d["cbc"])
            nc.scalar.dma_start(cwT[:], d["cwT"])
            nc.scalar.dma_start(w4sT[:], d["w4sT"])
            nc.scalar.dma_start(i768[:], d["i768"])
            nc.scalar.dma_start(b4r[0:1, :], d["b4r"])
            nc.vector.memset(ones[:], 1.0)

            # ================= Token stage =================
            with (
                tc.tile_pool(name="tok_sbuf", bufs=2) as tp,
                tc.tile_pool(name="tok_psum", bufs=1, space="PSUM") as pp,
            ):
                for b in range(BL):
                    r0 = b * P
                    xb = tp.tile([128, 2, C], F16, tag="xb")
                    nc.sync.dma_start(xb[:, 0, :], d["x16"][r0 : r0 + 128, :])
                    nc.sync.dma_start(xb[0:68, 1, :], d["x16"][r0 + 128 : r0 + P, :])

                    # x-norm broadcast tile: xnb[q, c] = sum_p x[p, c]^2
                    xsq = tp.tile([128, 2, C], F16, tag="xsq")
                    nc.vector.tensor_mul(xsq[:, 0, :], xb[:, 0, :], xb[:, 0, :])
                    nc.vector.tensor_mul(
                        xsq[0:68, 1, :], xb[0:68, 1, :], xb[0:68, 1, :]
                    )
                    ps_xnb = pp.tile([128, C], F32, tag="ps_xnb", bufs=1)
                    for no, nn_ in _n_slices(C):
                        nc.tensor.matmul(
                            ps_xnb[:, no : no + nn_],
                            ones[:, :],
                            xsq[:, 0, no : no + nn_],
                            start=True,
                            stop=False,
                        )
                        nc.tensor.matmul(
                            ps_xnb[:, no : no + nn_],
                            ones[0:68, :],
                            xsq[0:68, 1, no : no + nn_],
                            start=False,
                            stop=True,
                        )
                    xnb = tp.tile([128, C], F32, tag="xnb")
                    nc.scalar.copy(xnb[:], ps_xnb[:])

                    h1 = tp.tile([128, 3, C], F16, tag="h1")
                    for tcn in range(3):
                        ps_dot1 = pp.tile([128, C], F32, tag="ps_dot1", bufs=2)
                        for kc, kn in ((0, 128), (1, 68)):
                            for no, nn_ in _n_slices(C):
                                nc.tensor.matmul(
                                    ps_dot1[:, no : no + nn_],
                                    twT[0:kn, kc, tcn * 128 : (tcn + 1) * 128],
                                    xb[0:kn, kc, no : no + nn_],
                                    start=(kc == 0),
                                    stop=(kc == 1),
                                )
                        den = tp.tile([128, C], F32, tag="den")
                        nc.vector.affine_then_add(
                            den[:], ps_dot1[:], xnb[:],
                            scale=-2.0, bias=wnt[:, tcn : tcn + 1],
                        )
                        rec = tp.tile([128, C], F32, tag="rec")
                        nc.vector.reciprocal_approx_fast(rec[:], den[:])
                        sq = tp.tile([128, C], F32, tag="sq")
                        nc.scalar.activation(
                            sq[:], ps_dot1[:], AF.Square, bias=tbc[:, tcn : tcn + 1]
                        )
                        nc.gpsimd.tensor_mul(h1[:, tcn, :], sq[:], rec[:])

                    # token linear + shortcut + bias -> x2T columns for batch b
                    for mc in range(6):
                        ps_x2 = pp.tile([128, P], F32, tag="ps_x2", bufs=2)
                        for kc in range(3):
                            nc.tensor.matmul(
                                ps_x2[:],
                                h1[:, kc, mc * 128 : (mc + 1) * 128],
                                w2sT[:, kc, :],
                                start=(kc == 0),
                                stop=False,
                            )
                        for kc, kn in ((0, 128), (1, 68)):
                            nc.tensor.matmul(
                                ps_x2[:],
                                xb[0:kn, kc, mc * 128 : (mc + 1) * 128],
                                i196[0:kn, kc, :],
                                start=False,
                                stop=False,
                            )
                        nc.tensor.matmul(
                            ps_x2[:],
                            ones[0:1, :],
                            b2r[0:1, :],
                            start=False,
                            stop=True,
                        )
                        nc.scalar.copy(x2T[:, mc, r0 : r0 + P], ps_x2[:])

            # ================= Channel-stage row norms =================
            with (
                tc.tile_pool(name="xn_sbuf", bufs=1) as xp,
                tc.tile_pool(name="xn_psum", bufs=1, space="PSUM") as xpp,
            ):
                ps_xn2 = xpp.tile([128, ROWS], F32)
                for kc in range(6):
                    x2sq = xp.tile([128, ROWS], F16, tag="x2sq", bufs=2)
                    nc.vector.tensor_mul(x2sq[:], x2T[:, kc, :], x2T[:, kc, :])
                    for no, nn_ in _n_slices(ROWS):
                        nc.tensor.matmul(
                            ps_xn2[:, no : no + nn_],
                            ones[:, :],
                            x2sq[:, no : no + nn_],
                            start=(kc == 0),
                            stop=(kc == 5),
                        )
                nc.scalar.copy(xn2b[:], ps_xn2[:])

            # ================= Channel stage =================
            with (
                tc.tile_pool(name="ch_sbuf", bufs=2) as chp,
                tc.tile_pool(name="ch_psum", bufs=1, space="PSUM") as cpp,
            ):
                for r0 in range(0, ROWS, RB):
                    rn = min(RB, ROWS - r0)
                    nsub = _ceil_div(rn, 128)
                    po = [
                        cpp.tile([128, C], F32, tag=f"po{s}", bufs=1, name=f"po{s}")
                        for s in range(nsub)
                    ]
                    for mc in range(24):
                        ps_d2 = cpp.tile([128, RB], F32, tag="ps_d2", bufs=4)
                        for kc in range(6):
                            nc.tensor.matmul(
                                ps_d2[:, 0:rn],
                                cwT[:, kc, mc * 128 : (mc + 1) * 128],
                                x2T[:, kc, r0 : r0 + rn],
                                start=(kc == 0),
                                stop=(kc == 5),
                            )
                        den2 = chp.tile([128, RB], F32, tag="den2", bufs=4)
                        nc.vector.affine_then_add(
                            den2[:, 0:rn], ps_d2[:, 0:rn], xn2b[:, r0 : r0 + rn],
                            scale=-2.0, bias=wnc[:, mc : mc + 1],
                        )
                        rec2 = chp.tile([128, RB], F32, tag="rec2", bufs=4)
                        nc.vector.reciprocal_approx_fast(rec2[:, 0:rn], den2[:, 0:rn])
                        sq2 = chp.tile([128, RB], F32, tag="sq2", bufs=4)
                        nc.scalar.activation(
                            sq2[:, 0:rn], ps_d2[:, 0:rn], AF.Square,
                            bias=cbc[:, mc : mc + 1],
                        )
                        h2 = chp.tile([128, RB], F16, tag="h2", bufs=4)
                        nc.gpsimd.tensor_mul(h2[:, 0:rn], sq2[:, 0:rn], rec2[:, 0:rn])

                        for s in range(nsub):
                            sn = min(128, rn - s * 128)
                            for no, nn_ in _n_slices(C):
                                nc.tensor.matmul(
                                    po[s][0:sn, no : no + nn_],
                                    h2[:, s * 128 : s * 128 + sn],
                                    w4sT[:, mc, no : no + nn_],
                                    start=(mc == 0),
                                    stop=False,
                                )
                    # shortcut (identity over c) + bias b4
                    for s in range(nsub):
                        sn = min(128, rn - s * 128)
                        rs = r0 + s * 128
                        for kc in range(6):
                            for no, nn_ in _n_slices(C):
                                nc.tensor.matmul(
                                    po[s][0:sn, no : no + nn_],
                                    x2T[:, kc, rs : rs + sn],
                                    i768[:, kc, no : no + nn_],
                                    start=False,
                                    stop=False,
                                )
                        for no, nn_ in _n_slices(C):
                            nc.tensor.matmul(
                                po[s][0:sn, no : no + nn_],
                                ones[0:1, 0:sn],
                                b4r[0:1, no : no + nn_],
                                start=False,
                                stop=True,
                            )
                        osb = chp.tile([128, C], F32, tag="osb", bufs=3)
                        nc.scalar.copy(osb[0:sn, :], po[s][0:sn, :])
                        nc.sync.dma_start(out_dram[rs : rs + sn, :], osb[0:sn, :])

    nc.compile()
    return nc


def _pack_kpn(w, n_chunks):
    """(K, N) fp32 -> (128, n_chunks, N) fp16 with zero padding of K."""
    k, n = w.shape
    out = np.zeros((n_chunks * 128, n), np.float16)
    out[:k] = w.astype(np.float16)
    return np.ascontiguousarray(
        out.reshape(n_chunks, 128, n).transpose(1, 0, 2)
    )


def _pack_col(v, n_chunks):
    """(K,) fp32 -> (128, n_chunks) fp32 column chunks."""
    out = np.zeros((n_chunks * 128,), np.float32)
    out[: v.shape[0]] = v.astype(np.float32)
    return np.ascontiguousarray(out.reshape(n_chunks, 128).T)


_PROGRAM = None


def _get_program():
    global _PROGRAM
    if _PROGRAM is None:
        _PROGRAM = build_program()
    return _PROGRAM


def kernel(x, tw, tb, t_alpha, w2, b2, cw, cb, c_alpha, w4, b4, _trace=False):
    x = np.asarray(x, np.float32)
    tw = np.asarray(tw, np.float32)
    tb = np.asarray(tb, np.float32)
    w2 = np.asarray(w2, np.float32)
    b2 = np.asarray(b2, np.float32)
    cw = np.asarray(cw, np.float32)
    cb = np.asarray(cb, np.float32)
    w4 = np.asarray(w4, np.float32)
    b4 = np.asarray(b4, np.float32)

    # YAT output scales (exactly as the reference computes them), folded into
    # the following linear layers' weights and biases' stays separate.
    scale_t = np.float32(np.sqrt(np.float32(T / np.log(T + 1.0)))) ** np.asarray(
        t_alpha, np.float32
    )[0]
    scale_c = np.float32(np.sqrt(np.float32(M3 / np.log(M3 + 1.0)))) ** np.asarray(
        c_alpha, np.float32
    )[0]
    w2s = (w2 * scale_t).astype(np.float32)   # (P, T)
    w4s = (w4 * scale_c).astype(np.float32)   # (C, M3)

    shared = {
        "twT": _pack_kpn(tw.T, 2),                       # (196,384) -> (128,2,384)
        "w2sT": _pack_kpn(w2s.T, 3),                     # (384,196) -> (128,3,196)
        "i196": _pack_kpn(np.eye(P, dtype=np.float32), 2),
        "b2r": b2.astype(np.float16).reshape(1, P),
        "cwT": _pack_kpn(cw.T, 6),                       # (768,3072)
        "w4sT": _pack_kpn(w4s.T, 24),                    # (3072,768)
        "i768": _pack_kpn(np.eye(C, dtype=np.float32), 6),
        "b4r": b4.astype(np.float16).reshape(1, C),
        "wnt": _pack_col((tw.astype(np.float32) ** 2).sum(1) + EPS, 3),
        "tbc": _pack_col(tb, 3),
        "wnc": _pack_col((cw.astype(np.float32) ** 2).sum(1) + EPS, 24),
        "cbc": _pack_col(cb, 24),
    }
    x16 = x.astype(np.float16).reshape(NCORES, ROWS, C)
    in_maps = [dict(shared, x16=np.ascontiguousarray(x16[c])) for c in range(NCORES)]

    nc = _get_program()
    kwargs = {}
    if _trace:
        import shutil

        shutil.rmtree("/tmp/bass_ntff", ignore_errors=True)
        import os

        os.makedirs("/tmp/bass_ntff", exist_ok=True)
        kwargs["tmpdir"] = "/tmp/bass_ntff"
    res = bass_utils.run_bass_kernel_spmd(
        nc, in_maps, core_ids=list(range(NCORES)), trace=_trace, **kwargs
    )
    out = np.concatenate([res.results[c]["out"] for c in range(NCORES)], axis=0)
    out = out.reshape(B, P, C).astype(np.float32)
    if _trace:
        kernel.last_results = res
    return out


# revision 15
# speedup vs baseline: 1.0603x; 1.0603x over previous
"""Trainium2 Bass kernel for the YAT MixerBlock (nn_MixerBlock_12524124635797).

Strategy: pure data-parallel over batch (64 -> 8 per core). Each core runs
the full mixer block for its 8 batch elements.

Per-core dataflow (all GEMMs fp16 inputs, fp32 PSUM accumulation):
  Token stage (per batch b, x_b is (196p, 768c)):
    dot1 (384t-part, 768c-free) = twT.T @ x_b            [PE]
    den  = wn_t[t] + xn[c] - 2*dot1 + eps                [DVE affine_then_add]
    rec  = 1/den                                         [DVE reciprocal_approx_fast]
    sq   = (dot1 + tb[t])^2                              [ACT Square, bias slot]
    h1   = sq * rec  (fp16)                              [GPSIMD mult; scale_t folded into w2]
    x2T (768c-part, 196p-free) = h1.T@w2sT + x_b.T@I196 + ones.T@b2row   [PE, shortcut+bias
                                                          folded in as extra K rows]
  Channel stage (rows = (b,p) flattened, 1568 per core):
    xn2b (128, rows) = ones.T @ (x2T*x2T)                [PE broadcast of row norms]
    for row-block rb, for m-chunk mc (24 chunks of 3072):
      dot2 (128m-part, rows-free) = cwT.T @ x2T          [PE]
      den2/rec2/sq2/h2 as above (wn_c, cb per-partition) [DVE/ACT/GPSIMD]
      out_psum(rows-part, 768c) += h2.T @ w4sT[mc]       [PE]
    out_psum += x2T.T @ I768 + ones.T @ b4row            [PE, shortcut+bias]
    out (rows, 768) fp32 -> DRAM                         [ACT copy + DMA]
"""

import numpy as np

import concourse.bass as bass
import concourse.bacc as bacc
import concourse.mybir as mybir
from concourse import bass_utils
from concourse import tile

F16 = mybir.dt.float16
F32 = mybir.dt.float32
AF = mybir.ActivationFunctionType

EPS = 0.1
B, P, C, T, M3 = 64, 196, 768, 384, 3072
NCORES = 8
BL = B // NCORES          # 8 batches per core
ROWS = BL * P             # 1568 rows per core
RB = 256                  # row-block size for the channel stage (2 psum chunks)


def _ceil_div(a, b):
    return (a + b - 1) // b


def _n_slices(n, step=512):
    """Split [0, n) into matmul-legal free-dim slices (<=512, bank-aligned)."""
    out = []
    o = 0
    while o < n:
        out.append((o, min(step, n - o)))
        o += step
    return out


def build_program():
    nc = bacc.Bacc(
        "TRN2",
        target_bir_lowering=False,
        debug=False,
        enable_asserts=False,
        num_devices=NCORES,
    )

    # ---- DRAM I/O ----
    d = {}
    d["x16"] = nc.dram_tensor("x16", [ROWS, C], F16, kind="ExternalInput").ap()
    d["twT"] = nc.dram_tensor("twT", [128, 2, T], F16, kind="ExternalInput").ap()
    d["w2sT"] = nc.dram_tensor("w2sT", [128, 3, P], F16, kind="ExternalInput").ap()
    d["i196"] = nc.dram_tensor("i196", [128, 2, P], F16, kind="ExternalInput").ap()
    d["b2r"] = nc.dram_tensor("b2r", [1, P], F16, kind="ExternalInput").ap()
    d["cwT"] = nc.dram_tensor("cwT", [128, 6, M3], F16, kind="ExternalInput").ap()
    d["w4sT"] = nc.dram_tensor("w4sT", [128, 24, C], F16, kind="ExternalInput").ap()
    d["i768"] = nc.dram_tensor("i768", [128, 6, C], F16, kind="ExternalInput").ap()
    d["b4r"] = nc.dram_tensor("b4r", [1, C], F16, kind="ExternalInput").ap()
    d["wnt"] = nc.dram_tensor("wnt", [128, 3], F32, kind="ExternalInput").ap()
    d["tbc"] = nc.dram_tensor("tbc", [128, 3], F32, kind="ExternalInput").ap()
    d["wnc"] = nc.dram_tensor("wnc", [128, 24], F32, kind="ExternalInput").ap()
    d["cbc"] = nc.dram_tensor("cbc", [128, 24], F32, kind="ExternalInput").ap()
    out_dram = nc.dram_tensor("out", [ROWS, C], F32, kind="ExternalOutput").ap()

    with tile.TileContext(nc) as tc:
        with tc.tile_pool(name="consts", bufs=1) as cp:
            # Resident constants / persistent activations.
            twT = cp.tile([128, 2, T], F16)
            w2sT = cp.tile([128, 3, P], F16)
            i196 = cp.tile([128, 2, P], F16)
            b2r = cp.tile([128, P], F16)
            cwT = cp.tile([128, 6, M3], F16)
            w4sT = cp.tile([128, 24, C], F16)
            i768 = cp.tile([128, 6, C], F16)
            b4r = cp.tile([128, C], F16)
            wnt = cp.tile([128, 3], F32)
            tbc = cp.tile([128, 3], F32)
            wnc = cp.tile([128, 24], F32)
            cbc = cp.tile([128, 24], F32)
            ones = cp.tile([128, 128], F16)
            x2T = cp.tile([128, 6, ROWS], F16)
            xn2b = cp.tile([128, ROWS], F32)

            # x input first (token stage's critical path), then small token
            # constants, all on the sync queue; the big channel weights go on
            # the scalar-engine HWDGE queue so they don't block the token
            # stage.
            xbs = []
            for b in range(BL):
                r0 = b * P
                xb = cp.tile([128, 2, C], F16, name=f"xb{b}")
                nc.sync.dma_start(xb[:, 0, :], d["x16"][r0 : r0 + 128, :])
                nc.sync.dma_start(xb[0:68, 1, :], d["x16"][r0 + 128 : r0 + P, :])
                xbs.append(xb)
            nc.sync.dma_start(twT[:], d["twT"])
            nc.sync.dma_start(w2sT[:], d["w2sT"])
            nc.sync.dma_start(i196[:], d["i196"])
            nc.sync.dma_start(b2r[0:1, :], d["b2r"])
            nc.sync.dma_start(wnt[:], d["wnt"])
            nc.sync.dma_start(tbc[:], d["tbc"])
            nc.sync.dma_start(wnc[:], d["wnc"])
            nc.sync.dma_start(cbc[:], d["cbc"])
            nc.scalar.dma_start(cwT[:], d["cwT"])
            nc.scalar.dma_start(w4sT[:], d["w4sT"])
            nc.scalar.dma_start(i768[:], d["i768"])
            nc.scalar.dma_start(b4r[0:1, :], d["b4r"])
            nc.vector.memset(ones[:], 1.0)

            # ================= Token stage =================
            with (
                tc.tile_pool(name="tok_sbuf", bufs=2) as tp,
                tc.tile_pool(name="tok_psum", bufs=1, space="PSUM") as pp,
            ):
                for b in range(BL):
                    r0 = b * P
                    xb = xbs[b]

                    # x-norm broadcast tile: xnb[q, c] = sum_p x[p, c]^2
                    xsq = tp.tile([128, 2, C], F16, tag="xsq")
                    nc.vector.tensor_mul(xsq[:, 0, :], xb[:, 0, :], xb[:, 0, :])
                    nc.vector.tensor_mul(
                        xsq[0:68, 1, :], xb[0:68, 1, :], xb[0:68, 1, :]
                    )
                    ps_xnb = pp.tile([128, C], F32, tag="ps_xnb", bufs=1)
                    for no, nn_ in _n_slices(C):
                        nc.tensor.matmul(
                            ps_xnb[:, no : no + nn_],
                            ones[:, :],
                            xsq[:, 0, no : no + nn_],
                            start=True,
                            stop=False,
                        )
                        nc.tensor.matmul(
                            ps_xnb[:, no : no + nn_],
                            ones[0:68, :],
                            xsq[0:68, 1, no : no + nn_],
                            start=False,
                            stop=True,
                        )
                    xnb = tp.tile([128, C], F32, tag="xnb")
                    nc.scalar.copy(xnb[:], ps_xnb[:])

                    h1 = tp.tile([128, 3, C], F16, tag="h1")
                    for tcn in range(3):
                        ps_dot1 = pp.tile([128, C], F32, tag="ps_dot1", bufs=2)
                        for kc, kn in ((0, 128), (1, 68)):
                            for no, nn_ in _n_slices(C):
                                nc.tensor.matmul(
                                    ps_dot1[:, no : no + nn_],
                                    twT[0:kn, kc, tcn * 128 : (tcn + 1) * 128],
                                    xb[0:kn, kc, no : no + nn_],
                                    start=(kc == 0),
                                    stop=(kc == 1),
                                )
                        den = tp.tile([128, C], F32, tag="den")
                        nc.vector.affine_then_add(
                            den[:], ps_dot1[:], xnb[:],
                            scale=-2.0, bias=wnt[:, tcn : tcn + 1],
                        )
                        rec = tp.tile([128, C], F32, tag="rec")
                        nc.vector.reciprocal_approx_fast(rec[:], den[:])
                        sq = tp.tile([128, C], F32, tag="sq")
                        nc.scalar.activation(
                            sq[:], ps_dot1[:], AF.Square, bias=tbc[:, tcn : tcn + 1]
                        )
                        nc.gpsimd.tensor_mul(h1[:, tcn, :], sq[:], rec[:])

                    # token linear + shortcut + bias -> x2T columns for batch b
                    for mc in range(6):
                        ps_x2 = pp.tile([128, P], F32, tag="ps_x2", bufs=2)
                        for kc in range(3):
                            nc.tensor.matmul(
                                ps_x2[:],
                                h1[:, kc, mc * 128 : (mc + 1) * 128],
                                w2sT[:, kc, :],
                                start=(kc == 0),
                                stop=False,
                            )
                        for kc, kn in ((0, 128), (1, 68)):
                            nc.tensor.matmul(
                                ps_x2[:],
                                xb[0:kn, kc, mc * 128 : (mc + 1) * 128],
                                i196[0:kn, kc, :],
                                start=False,
                                stop=False,
                            )
                        nc.tensor.matmul(
                            ps_x2[:],
                            ones[0:1, :],
                            b2r[0:1, :],
                            start=False,
                            stop=True,
                        )
                        nc.scalar.copy(x2T[:, mc, r0 : r0 + P], ps_x2[:])

            # ================= Channel-stage row norms =================
            with (
                tc.tile_pool(name="xn_sbuf", bufs=1) as xp,
                tc.tile_pool(name="xn_psum", bufs=1, space="PSUM") as xpp,
            ):
                ps_xn2 = xpp.tile([128, ROWS], F32)
                for kc in range(6):
                    x2sq = xp.tile([128, ROWS], F16, tag="x2sq", bufs=2)
                    nc.vector.tensor_mul(x2sq[:], x2T[:, kc, :], x2T[:, kc, :])
                    for no, nn_ in _n_slices(ROWS):
                        nc.tensor.matmul(
                            ps_xn2[:, no : no + nn_],
                            ones[:, :],
                            x2sq[:, no : no + nn_],
                            start=(kc == 0),
                            stop=(kc == 5),
                        )
                nc.scalar.copy(xn2b[:], ps_xn2[:])

            # ================= Channel stage =================
            with (
                tc.tile_pool(name="ch_sbuf", bufs=2) as chp,
                tc.tile_pool(name="ch_psum", bufs=1, space="PSUM") as cpp,
            ):
                for r0 in range(0, ROWS, RB):
                    rn = min(RB, ROWS - r0)
                    nsub = _ceil_div(rn, 128)
                    po = [
                        cpp.tile([128, C], F32, tag=f"po{s}", bufs=1, name=f"po{s}")
                        for s in range(nsub)
                    ]
                    for mc in range(24):
                        ps_d2 = cpp.tile([128, RB], F32, tag="ps_d2", bufs=4)
                        for kc in range(6):
                            nc.tensor.matmul(
                                ps_d2[:, 0:rn],
                                cwT[:, kc, mc * 128 : (mc + 1) * 128],
                                x2T[:, kc, r0 : r0 + rn],
                                start=(kc == 0),
                                stop=(kc == 5),
                            )
                        den2 = chp.tile([128, RB], F32, tag="den2", bufs=4)
                        nc.vector.affine_then_add(
                            den2[:, 0:rn], ps_d2[:, 0:rn], xn2b[:, r0 : r0 + rn],
                            scale=-2.0, bias=wnc[:, mc : mc + 1],
                        )
                        rec2 = chp.tile([128, RB], F32, tag="rec2", bufs=4)
                        nc.vector.reciprocal_approx_fast(rec2[:, 0:rn], den2[:, 0:rn])
                        sq2 = chp.tile([128, RB], F32, tag="sq2", bufs=4)
                        nc.scalar.activation(
                            sq2[:, 0:rn], ps_d2[:, 0:rn], AF.Square,
                            bias=cbc[:, mc : mc + 1],
                        )
                        h2 = chp.tile([128, RB], F16, tag="h2", bufs=4)
                        nc.gpsimd.tensor_mul(h2[:, 0:rn], sq2[:, 0:rn], rec2[:, 0:rn])

                        for s in range(nsub):
                            sn = min(128, rn - s * 128)
                            for no, nn_ in _n_slices(C):
                                nc.tensor.matmul(
                                    po[s][0:sn, no : no + nn_],
                                    h2[:, s * 128 : s * 128 + sn],
                                    w4sT[:, mc, no : no + nn_],
                                    start=(mc == 0),
                                    stop=False,
                                )
                    # shortcut (identity over c) + bias b4
                    for s in range(nsub):
                        sn = min(128, rn - s * 128)
                        rs = r0 + s * 128
                        for kc in range(6):
                            for no, nn_ in _n_slices(C):
                                nc.tensor.matmul(
                                    po[s][0:sn, no : no + nn_],
                                    x2T[:, kc, rs : rs + sn],
                                    i768[:, kc, no : no + nn_],
                                    start=False,
                                    stop=False,
                                )
                        for no, nn_ in _n_slices(C):
                            nc.tensor.matmul(
                                po[s][0:sn, no : no + nn_],
                                ones[0:1, 0:sn],
                                b4r[0:1, no : no + nn_],
                                start=False,
                                stop=True,
                            )
                        osb = chp.tile([128, C], F32, tag="osb", bufs=3)
                        nc.scalar.copy(osb[0:sn, :], po[s][0:sn, :])
                        nc.sync.dma_start(out_dram[rs : rs + sn, :], osb[0:sn, :])

    nc.compile()
    return nc


def _pack_kpn(w, n_chunks):
    """(K, N) fp32 -> (128, n_chunks, N) fp16 with zero padding of K."""
    k, n = w.shape
    out = np.zeros((n_chunks * 128, n), np.float16)
    out[:k] = w.astype(np.float16)
    return np.ascontiguousarray(
        out.reshape(n_chunks, 128, n).transpose(1, 0, 2)
    )


def _pack_col(v, n_chunks):
    """(K,) fp32 -> (128, n_chunks) fp32 column chunks."""
    out = np.zeros((n_chunks * 128,), np.float32)
    out[: v.shape[0]] = v.astype(np.float32)
    return np.ascontiguousarray(out.reshape(n_chunks, 128).T)


_PROGRAM = None


def _get_program():
    global _PROGRAM
    if _PROGRAM is None:
        _PROGRAM = build_program()
    return _PROGRAM


def kernel(x, tw, tb, t_alpha, w2, b2, cw, cb, c_alpha, w4, b4, _trace=False):
    x = np.asarray(x, np.float32)
    tw = np.asarray(tw, np.float32)
    tb = np.asarray(tb, np.float32)
    w2 = np.asarray(w2, np.float32)
    b2 = np.asarray(b2, np.float32)
    cw = np.asarray(cw, np.float32)
    cb = np.asarray(cb, np.float32)
    w4 = np.asarray(w4, np.float32)
    b4 = np.asarray(b4, np.float32)

    # YAT output scales (exactly as the reference computes them), folded into
    # the following linear layers' weights and biases' stays separate.
    scale_t = np.float32(np.sqrt(np.float32(T / np.log(T + 1.0)))) ** np.asarray(
        t_alpha, np.float32
    )[0]
    scale_c = np.float32(np.sqrt(np.float32(M3 / np.log(M3 + 1.0)))) ** np.asarray(
        c_alpha, np.float32
    )[0]
    w2s = (w2 * scale_t).astype(np.float32)   # (P, T)
    w4s = (w4 * scale_c).astype(np.float32)   # (C, M3)

    shared = {
        "twT": _pack_kpn(tw.T, 2),                       # (196,384) -> (128,2,384)
        "w2sT": _pack_kpn(w2s.T, 3),                     # (384,196) -> (128,3,196)
        "i196": _pack_kpn(np.eye(P, dtype=np.float32), 2),
        "b2r": b2.astype(np.float16).reshape(1, P),
        "cwT": _pack_kpn(cw.T, 6),                       # (768,3072)
        "w4sT": _pack_kpn(w4s.T, 24),                    # (3072,768)
        "i768": _pack_kpn(np.eye(C, dtype=np.float32), 6),
        "b4r": b4.astype(np.float16).reshape(1, C),
        "wnt": _pack_col((tw.astype(np.float32) ** 2).sum(1) + EPS, 3),
        "tbc": _pack_col(tb, 3),
        "wnc": _pack_col((cw.astype(np.float32) ** 2).sum(1) + EPS, 24),
        "cbc": _pack_col(cb, 24),
    }
    x16 = x.astype(np.float16).reshape(NCORES, ROWS, C)
    in_maps = [dict(shared, x16=np.ascontiguousarray(x16[c])) for c in range(NCORES)]

    nc = _get_program()
    kwargs = {}
    if _trace:
        import shutil

        shutil.rmtree("/tmp/bass_ntff", ignore_errors=True)
        import os

        os.makedirs("/tmp/bass_ntff", exist_ok=True)
        kwargs["tmpdir"] = "/tmp/bass_ntff"
    res = bass_utils.run_bass_kernel_spmd(
        nc, in_maps, core_ids=list(range(NCORES)), trace=_trace, **kwargs
    )
    out = np.concatenate([res.results[c]["out"] for c in range(NCORES)], axis=0)
    out = out.reshape(B, P, C).astype(np.float32)
    if _trace:
        kernel.last_results = res
    return out


# revision 26
# speedup vs baseline: 1.1783x; 1.1113x over previous
"""Trainium2 Bass kernel for the YAT MixerBlock (nn_MixerBlock_12524124635797).

Strategy: pure data-parallel over batch (64 -> 8 per core). Each core runs
the full mixer block for its 8 batch elements.

Per-core dataflow (all GEMMs fp16 inputs, fp32 PSUM accumulation):
  Token stage (per batch b, x_b is (196p, 768c)):
    dot1 (384t-part, 768c-free) = twT.T @ x_b            [PE]
    den  = wn_t[t] + xn[c] - 2*dot1 + eps                [DVE affine_then_add]
    rec  = 1/den                                         [DVE reciprocal_approx_fast]
    sq   = (dot1 + tb[t])^2                              [ACT Square, bias slot]
    h1   = sq * rec  (fp16)                              [GPSIMD mult; scale_t folded into w2]
    x2T (768c-part, 196p-free) = h1.T@w2sT + x_b.T@I196 + ones.T@b2row   [PE, shortcut+bias
                                                          folded in as extra K rows]
  Channel stage (rows = (b,p) flattened, 1568 per core):
    xn2b (128, rows) = ones.T @ (x2T*x2T)                [PE broadcast of row norms]
    for row-block rb, for m-chunk mc (24 chunks of 3072):
      dot2 (128m-part, rows-free) = cwT.T @ x2T          [PE]
      den2/rec2/sq2/h2 as above (wn_c, cb per-partition) [DVE/ACT/GPSIMD]
      out_psum(rows-part, 768c) += h2.T @ w4sT[mc]       [PE]
    out_psum += x2T.T @ I768 + ones.T @ b4row            [PE, shortcut+bias]
    out (rows, 768) fp32 -> DRAM                         [ACT copy + DMA]
"""

import numpy as np

import concourse.bass as bass
import concourse.bacc as bacc
import concourse.mybir as mybir
from concourse import bass_utils
from concourse import tile

F16 = mybir.dt.float16
F32 = mybir.dt.float32
AF = mybir.ActivationFunctionType

EPS = 0.1
B, P, C, T, M3 = 64, 196, 768, 384, 3072
NCORES = 8
BL = B // NCORES          # 8 batches per core
ROWS = BL * P             # 1568 rows per core
ROWSP = 1664              # ROWS padded to a multiple of 128
RB = 256                  # row-block size for the channel stage (2 psum chunks)


def _ceil_div(a, b):
    return (a + b - 1) // b


def _n_slices(n, step=512):
    """Split [0, n) into matmul-legal free-dim slices (<=512, bank-aligned)."""
    out = []
    o = 0
    while o < n:
        out.append((o, min(step, n - o)))
        o += step
    return out


def build_program():
    nc = bacc.Bacc(
        "TRN2",
        target_bir_lowering=False,
        debug=False,
        enable_asserts=False,
        num_devices=NCORES,
    )

    # ---- DRAM I/O ----
    d = {}
    d["x16"] = nc.dram_tensor("x16", [ROWS, C], F16, kind="ExternalInput").ap()
    d["twT"] = nc.dram_tensor("twT", [128, 2, T], F16, kind="ExternalInput").ap()
    d["w2sT"] = nc.dram_tensor("w2sT", [128, 3, P], F16, kind="ExternalInput").ap()
    d["i196"] = nc.dram_tensor("i196", [128, 2, P], F16, kind="ExternalInput").ap()
    d["b2r"] = nc.dram_tensor("b2r", [1, P], F16, kind="ExternalInput").ap()
    d["cwT"] = nc.dram_tensor("cwT", [128, 6, M3], F16, kind="ExternalInput").ap()
    d["w4sT"] = nc.dram_tensor("w4sT", [128, 24, C], F16, kind="ExternalInput").ap()
    d["b4r"] = nc.dram_tensor("b4r", [1, C], F16, kind="ExternalInput").ap()
    d["wnt"] = nc.dram_tensor("wnt", [128, 3], F32, kind="ExternalInput").ap()
    d["tbc"] = nc.dram_tensor("tbc", [128, 3], F32, kind="ExternalInput").ap()
    d["wnc"] = nc.dram_tensor("wnc", [128, 24], F32, kind="ExternalInput").ap()
    d["cbc"] = nc.dram_tensor("cbc", [128, 24], F32, kind="ExternalInput").ap()
    out_dram = nc.dram_tensor("out", [ROWS, C], F32, kind="ExternalOutput").ap()

    with tile.TileContext(nc) as tc:
        with tc.tile_pool(name="consts", bufs=1) as cp:
            # Resident constants / persistent activations.
            twT = cp.tile([128, 2, T], F16)
            w2sT = cp.tile([128, 3, P], F16)
            i196 = cp.tile([128, 2, P], F16)
            b2r = cp.tile([128, P], F16)
            cwT = cp.tile([128, 6, M3], F16)
            w4sT = cp.tile([128, 24, C], F16)
            b4r = cp.tile([128, C], F16)
            wnt = cp.tile([128, 3], F32)
            tbc = cp.tile([128, 3], F32)
            wnc = cp.tile([128, 24], F32)
            cbc = cp.tile([128, 24], F32)
            ones = cp.tile([128, 128], F16)
            # Free dim padded to a multiple of 128 so the tail row-block's
            # 128-col DMA transpose reads stay in bounds (garbage cols unused).
            x2T = cp.tile([128, 6, ROWSP], F16)
            xn2b = cp.tile([128, ROWS], F32)

            # x input first (token stage's critical path), then small token
            # constants, all on the sync queue; the big channel weights go on
            # the scalar-engine HWDGE queue so they don't block the token
            # stage.
            xbs = []
            for b in range(BL):
                r0 = b * P
                xb = cp.tile([128, 2, C], F16, name=f"xb{b}")
                nc.sync.dma_start(xb[:, 0, :], d["x16"][r0 : r0 + 128, :])
                nc.sync.dma_start(xb[0:68, 1, :], d["x16"][r0 + 128 : r0 + P, :])
                xbs.append(xb)
            nc.sync.dma_start(twT[:], d["twT"])
            nc.sync.dma_start(w2sT[:], d["w2sT"])
            nc.sync.dma_start(i196[:], d["i196"])
            nc.sync.dma_start(b2r[0:1, :], d["b2r"])
            nc.sync.dma_start(wnt[:], d["wnt"])
            nc.sync.dma_start(tbc[:], d["tbc"])
            nc.sync.dma_start(wnc[:], d["wnc"])
            nc.sync.dma_start(cbc[:], d["cbc"])
            nc.scalar.dma_start(cwT[:], d["cwT"])
            nc.scalar.dma_start(w4sT[:], d["w4sT"])
            nc.scalar.dma_start(b4r[0:1, :], d["b4r"])
            nc.vector.memset(ones[:], 1.0)
            nc.vector.memset(x2T[:, :, ROWS:ROWSP], 0.0)

            # ================= Token stage =================
            with (
                tc.tile_pool(name="tok_sbuf", bufs=2) as tp,
                tc.tile_pool(name="tok_psum", bufs=1, space="PSUM") as pp,
            ):
                for b in range(BL):
                    r0 = b * P
                    xb = xbs[b]

                    # x-norm broadcast tile: xnb[q, c] = sum_p x[p, c]^2
                    xsq = tp.tile([128, 2, C], F16, tag="xsq")
                    nc.vector.tensor_mul(xsq[:, 0, :], xb[:, 0, :], xb[:, 0, :])
                    nc.vector.tensor_mul(
                        xsq[0:68, 1, :], xb[0:68, 1, :], xb[0:68, 1, :]
                    )
                    ps_xnb = pp.tile([128, C], F32, tag="ps_xnb", bufs=1)
                    for no, nn_ in _n_slices(C):
                        nc.tensor.matmul(
                            ps_xnb[:, no : no + nn_],
                            ones[:, :],
                            xsq[:, 0, no : no + nn_],
                            start=True,
                            stop=False,
                        )
                        nc.tensor.matmul(
                            ps_xnb[:, no : no + nn_],
                            ones[0:68, :],
                            xsq[0:68, 1, no : no + nn_],
                            start=False,
                            stop=True,
                        )
                    xnb = tp.tile([128, C], F32, tag="xnb")
                    nc.scalar.copy(xnb[:], ps_xnb[:])

                    h1 = tp.tile([128, 3, C], F16, tag="h1")
                    for tcn in range(3):
                        ps_dot1 = pp.tile([128, C], F32, tag="ps_dot1", bufs=2)
                        for kc, kn in ((0, 128), (1, 68)):
                            for no, nn_ in _n_slices(C):
                                nc.tensor.matmul(
                                    ps_dot1[:, no : no + nn_],
                                    twT[0:kn, kc, tcn * 128 : (tcn + 1) * 128],
                                    xb[0:kn, kc, no : no + nn_],
                                    start=(kc == 0),
                                    stop=(kc == 1),
                                )
                        den = tp.tile([128, C], F32, tag="den")
                        nc.vector.affine_then_add(
                            den[:], ps_dot1[:], xnb[:],
                            scale=-2.0, bias=wnt[:, tcn : tcn + 1],
                        )
                        rec = tp.tile([128, C], F32, tag="rec")
                        nc.vector.reciprocal_approx_fast(rec[:], den[:])
                        sq = tp.tile([128, C], F32, tag="sq")
                        nc.scalar.activation(
                            sq[:], ps_dot1[:], AF.Square, bias=tbc[:, tcn : tcn + 1]
                        )
                        nc.gpsimd.tensor_mul(h1[:, tcn, :], sq[:], rec[:])

                    # token linear + shortcut + bias -> x2T columns for batch b
                    for mc in range(6):
                        ps_x2 = pp.tile([128, P], F32, tag="ps_x2", bufs=2)
                        for kc in range(3):
                            nc.tensor.matmul(
                                ps_x2[:],
                                h1[:, kc, mc * 128 : (mc + 1) * 128],
                                w2sT[:, kc, :],
                                start=(kc == 0),
                                stop=False,
                            )
                        for kc, kn in ((0, 128), (1, 68)):
                            nc.tensor.matmul(
                                ps_x2[:],
                                xb[0:kn, kc, mc * 128 : (mc + 1) * 128],
                                i196[0:kn, kc, :],
                                start=False,
                                stop=False,
                            )
                        nc.tensor.matmul(
                            ps_x2[:],
                            ones[0:1, :],
                            b2r[0:1, :],
                            start=False,
                            stop=True,
                        )
                        nc.scalar.copy(x2T[:, mc, r0 : r0 + P], ps_x2[:])

            # ================= Channel-stage row norms =================
            with (
                tc.tile_pool(name="xn_sbuf", bufs=1) as xp,
                tc.tile_pool(name="xn_psum", bufs=1, space="PSUM") as xpp,
            ):
                ps_xn2 = xpp.tile([128, ROWS], F32)
                for kc in range(6):
                    x2sq = xp.tile([128, ROWS], F16, tag="x2sq", bufs=2)
                    nc.vector.tensor_mul(x2sq[:], x2T[:, kc, 0:ROWS], x2T[:, kc, 0:ROWS])
                    for no, nn_ in _n_slices(ROWS):
                        nc.tensor.matmul(
                            ps_xn2[:, no : no + nn_],
                            ones[:, :],
                            x2sq[:, no : no + nn_],
                            start=(kc == 0),
                            stop=(kc == 5),
                        )
                nc.scalar.copy(xn2b[:], ps_xn2[:])

            # ================= Channel stage =================
            with (
                tc.tile_pool(name="ch_sbuf", bufs=2) as chp,
                tc.tile_pool(name="ch_psum", bufs=1, space="PSUM") as cpp,
            ):
                for r0 in range(0, ROWS, RB):
                    rn = min(RB, ROWS - r0)
                    nsub = _ceil_div(rn, 128)
                    po = [
                        cpp.tile([128, C], F32, tag=f"po{s}", bufs=1, name=f"po{s}")
                        for s in range(nsub)
                    ]
                    for mc in range(24):
                        ps_d2 = cpp.tile([128, RB], F32, tag="ps_d2", bufs=4)
                        for kc in range(6):
                            nc.tensor.matmul(
                                ps_d2[:, 0:rn],
                                cwT[:, kc, mc * 128 : (mc + 1) * 128],
                                x2T[:, kc, r0 : r0 + rn],
                                start=(kc == 0),
                                stop=(kc == 5),
                            )
                        den2 = chp.tile([128, RB], F32, tag="den2", bufs=4)
                        nc.vector.affine_then_add(
                            den2[:, 0:rn], ps_d2[:, 0:rn], xn2b[:, r0 : r0 + rn],
                            scale=-2.0, bias=wnc[:, mc : mc + 1],
                        )
                        rec2 = chp.tile([128, RB], F32, tag="rec2", bufs=4)
                        nc.vector.reciprocal_approx_fast(rec2[:, 0:rn], den2[:, 0:rn])
                        sq2 = chp.tile([128, RB], F32, tag="sq2", bufs=4)
                        nc.scalar.activation(
                            sq2[:, 0:rn], ps_d2[:, 0:rn], AF.Square,
                            bias=cbc[:, mc : mc + 1],
                        )
                        h2 = chp.tile([128, RB], F16, tag="h2", bufs=4)
                        # Alternate the multiply between GPSIMD and DVE so the
                        # last link of the yat chain isn't serialized on one
                        # engine's FIFO.
                        mul_eng = nc.gpsimd if mc % 3 else nc.vector
                        mul_eng.tensor_mul(h2[:, 0:rn], sq2[:, 0:rn], rec2[:, 0:rn])

                        for s in range(nsub):
                            sn = min(128, rn - s * 128)
                            for no, nn_ in _n_slices(C):
                                nc.tensor.matmul(
                                    po[s][0:sn, no : no + nn_],
                                    h2[:, s * 128 : s * 128 + sn],
                                    w4sT[:, mc, no : no + nn_],
                                    start=(mc == 0),
                                    stop=False,
                                )
                    # bias b4 row, then shortcut x2 added via DVE from a
                    # DMA-transposed copy of x2T (cheaper than routing the
                    # identity through the PE).
                    for s in range(nsub):
                        sn = min(128, rn - s * 128)
                        rs = r0 + s * 128
                        for no, nn_ in _n_slices(C):
                            nc.tensor.matmul(
                                po[s][0:sn, no : no + nn_],
                                ones[0:1, 0:sn],
                                b4r[0:1, no : no + nn_],
                                start=False,
                                stop=True,
                            )
                        x2row = chp.tile([128, 6, 128], F16, tag="x2row", bufs=3)
                        for kc in range(6):
                            # Always a full 128-col source block (x2T free dim
                            # is padded); extra rows of x2row are unused.
                            nc.sync.dma_start_transpose(
                                x2row[:, kc, :], x2T[:, kc, rs : rs + 128]
                            )
                        osb = chp.tile([128, C], F32, tag="osb", bufs=3)
                        nc.vector.tensor_add(
                            osb[0:sn, :],
                            po[s][0:sn, :],
                            x2row[0:sn, :, :].rearrange("p a b -> p (a b)"),
                        )
                        nc.sync.dma_start(out_dram[rs : rs + sn, :], osb[0:sn, :])

    nc.compile()
    return nc


def _pack_kpn(w, n_chunks):
    """(K, N) fp32 -> (128, n_chunks, N) fp16 with zero padding of K."""
    k, n = w.shape
    out = np.zeros((n_chunks * 128, n), np.float16)
    out[:k] = w.astype(np.float16)
    return np.ascontiguousarray(
        out.reshape(n_chunks, 128, n).transpose(1, 0, 2)
    )


def _pack_col(v, n_chunks):
    """(K,) fp32 -> (128, n_chunks) fp32 column chunks."""
    out = np.zeros((n_chunks * 128,), np.float32)
    out[: v.shape[0]] = v.astype(np.float32)
    return np.ascontiguousarray(out.reshape(n_chunks, 128).T)


_PROGRAM = None


def _get_program():
    global _PROGRAM
    if _PROGRAM is None:
        _PROGRAM = build_program()
    return _PROGRAM


def kernel(x, tw, tb, t_alpha, w2, b2, cw, cb, c_alpha, w4, b4, _trace=False):
    x = np.asarray(x, np.float32)
    tw = np.asarray(tw, np.float32)
    tb = np.asarray(tb, np.float32)
    w2 = np.asarray(w2, np.float32)
    b2 = np.asarray(b2, np.float32)
    cw = np.asarray(cw, np.float32)
    cb = np.asarray(cb, np.float32)
    w4 = np.asarray(w4, np.float32)
    b4 = np.asarray(b4, np.float32)

    # YAT output scales (exactly as the reference computes them), folded into
    # the following linear layers' weights and biases' stays separate.
    scale_t = np.float32(np.sqrt(np.float32(T / np.log(T + 1.0)))) ** np.asarray(
        t_alpha, np.float32
    )[0]
    scale_c = np.float32(np.sqrt(np.float32(M3 / np.log(M3 + 1.0)))) ** np.asarray(
        c_alpha, np.float32
    )[0]
    w2s = (w2 * scale_t).astype(np.float32)   # (P, T)
    w4s = (w4 * scale_c).astype(np.float32)   # (C, M3)

    shared = {
        "twT": _pack_kpn(tw.T, 2),                       # (196,384) -> (128,2,384)
        "w2sT": _pack_kpn(w2s.T, 3),                     # (384,196) -> (128,3,196)
        "i196": _pack_kpn(np.eye(P, dtype=np.float32), 2),
        "b2r": b2.astype(np.float16).reshape(1, P),
        "cwT": _pack_kpn(cw.T, 6),                       # (768,3072)
        "w4sT": _pack_kpn(w4s.T, 24),                    # (3072,768)
        "b4r": b4.astype(np.float16).reshape(1, C),
        "wnt": _pack_col((tw.astype(np.float32) ** 2).sum(1) + EPS, 3),
        "tbc": _pack_col(tb, 3),
        "wnc": _pack_col((cw.astype(np.float32) ** 2).sum(1) + EPS, 24),
        "cbc": _pack_col(cb, 24),
    }
    x16 = x.astype(np.float16).reshape(NCORES, ROWS, C)
    in_maps = [dict(shared, x16=np.ascontiguousarray(x16[c])) for c in range(NCORES)]

    nc = _get_program()
    kwargs = {}
    if _trace:
        import shutil

        shutil.rmtree("/tmp/bass_ntff", ignore_errors=True)
        import os

        os.makedirs("/tmp/bass_ntff", exist_ok=True)
        kwargs["tmpdir"] = "/tmp/bass_ntff"
    res = bass_utils.run_bass_kernel_spmd(
        nc, in_maps, core_ids=list(range(NCORES)), trace=_trace, **kwargs
    )
    out = np.concatenate([res.results[c]["out"] for c in range(NCORES)], axis=0)
    out = out.reshape(B, P, C).astype(np.float32)
    if _trace:
        kernel.last_results = res
    return out
